# revision 3
# baseline (speedup 1.0000x reference)
"""Trainium2 Bass kernel for LoRA linear: y = x @ (W + 2*B@A).T + b.

Full inputs: x (8, 2048, 2048) f32, W (2048, 2048) f32, b (2048,) f32,
B (2048, 16) f32, A (16, 2048) f32.  Output (8, 2048, 2048) f32.

Sharding: data-parallel over the batch dim — core i computes
y[i] = x[i] @ w.T + b with the merged weight w = W + 2*B@A.

Per-core kernel (all compute in bf16 on the TensorEngine, f32 accumulate):
  phase 0: load A, gather 2*B.T, broadcast bias, build identity.
  phase 1: build wT[d, o] = W.T + A.T @ (2B).T in PSUM
           (rank-16 matmul clears the bank, 4 PE transposes of W tiles
           accumulate on top), copy-cast to bf16 SBUF.
  phase 2: per 128-row x tile: PE-transpose x into bf16 lhsT tiles, then
           16x [128,128]x[128,512] bf16 matmuls per output bank,
           DVE adds the bias during PSUM->SBUF eviction, DMA out.
"""

import numpy as np

import concourse.bacc as bacc
import concourse.mybir as mybir
import concourse.tile as tile
from concourse import masks
from concourse.bass_utils import run_bass_kernel_spmd

N_CORES = 8
BATCH, S, D = 8, 2048, 2048
RANK = 16
SCALE = 2.0  # alpha / rank = 32 / 16
P = 128  # partitions
FREE = 512  # f32 elems per PSUM bank
ND = D // P  # 16 contraction tiles
NS = S // P  # 16 row tiles per core
NO = D // FREE  # 4 output banks per row tile
NG = ND // 4  # 4 transpose groups (4x 128-col transposes per PSUM bank)

F32 = mybir.dt.float32
BF16 = mybir.dt.bfloat16


def build_nc():
    nc = bacc.Bacc(
        "TRN2", target_bir_lowering=False, debug=False, num_devices=N_CORES
    )
    x_d = nc.dram_tensor("x", [S, D], F32, kind="ExternalInput").ap()
    W_d = nc.dram_tensor("W", [D, D], F32, kind="ExternalInput").ap()
    b_d = nc.dram_tensor("b", [D], F32, kind="ExternalInput").ap()
    B_d = nc.dram_tensor("B", [D, RANK], F32, kind="ExternalInput").ap()
    A_d = nc.dram_tensor("A", [RANK, D], F32, kind="ExternalInput").ap()
    out_d = nc.dram_tensor("out", [S, D], F32, kind="ExternalOutput").ap()

    with tile.TileContext(nc) as tc:
        with (
            tc.tile_pool(name="singles", bufs=1) as singles,
            tc.tile_pool(name="wt", bufs=1) as wtp,
        ):
            ident = singles.tile([P, P], F32)
            masks.make_identity(nc, ident[:])

            # bias replicated across all 128 partitions
            bb = singles.tile([P, D], F32)
            nc.gpsimd.dma_start(out=bb[:], in_=b_d[None, :].broadcast_to([P, D]))

            A_sb = singles.tile([RANK, D], F32)
            nc.sync.dma_start(out=A_sb[:], in_=A_d[:])

            # 2 * B.T: load B as [128, (t, r)] then PE-transpose per 128-row tile
            B2T = singles.tile([RANK, D], F32)
            Bs = singles.tile([P, ND * RANK], F32)
            nc.sync.dma_start(
                out=Bs[:], in_=B_d.rearrange("(t p) r -> p t r", p=P)
            )

            # merged transposed weight, bf16: wT[p, dt, o] = w[o, dt*128+p]
            wT = wtp.tile([P, ND, D], BF16)

            # ---- phase 1: build wT = W.T + A.T @ (2B).T ----
            with (
                tc.tile_pool(name="wstage", bufs=5) as wstage,
                tc.tile_pool(name="bpsum", bufs=4, space="PSUM") as bpsum,
            ):
                # build 2*B.T from the staged B tiles
                for g in range(NG):
                    bps = bpsum.tile([RANK, 4 * P], F32, tag="btp")
                    for j in range(4):
                        t = 4 * g + j
                        nc.tensor.matmul(
                            bps[:, j * P : (j + 1) * P],
                            Bs[:, t * RANK : (t + 1) * RANK],
                            ident[:],
                            is_transpose=True,
                            start=(j == 0),
                            stop=(j == 3),
                        )
                    nc.vector.tensor_scalar_mul(
                        B2T[:, g * 4 * P : (g + 1) * 4 * P], bps[:], SCALE
                    )
                for g in range(NG):
                    wst = []
                    for j in range(4):
                        ot = 4 * g + j
                        w_tile = wstage.tile([P, D], F32, tag="wst")
                        nc.sync.dma_start(
                            out=w_tile[:], in_=W_d[ot * P : (ot + 1) * P, :]
                        )
                        wst.append(w_tile)
                    for dt in range(ND):
                        ps = bpsum.tile([P, FREE], F32)
                        # rank-16 LoRA delta fills the whole bank (start=True)
                        nc.tensor.matmul(
                            ps[:],
                            A_sb[:, dt * P : (dt + 1) * P],
                            B2T[:, g * FREE : (g + 1) * FREE],
                            start=True,
                            stop=False,
                        )
                        # W.T tiles accumulate on top of the delta
                        for j in range(4):
                            nc.tensor.matmul(
                                ps[:, j * P : (j + 1) * P],
                                wst[j][:, dt * P : (dt + 1) * P],
                                ident[:],
                                is_transpose=True,
                                start=False,
                                stop=(j == 3),
                            )
                        nc.vector.tensor_copy(
                            wT[:, dt, g * FREE : (g + 1) * FREE], ps[:]
                        )

            # ---- phase 2: y = x @ wT + b ----
            with (
                tc.tile_pool(name="xstage", bufs=3) as xstage,
                tc.tile_pool(name="xTp", bufs=2) as xTp,
                tc.tile_pool(name="yout", bufs=3) as youtp,
                tc.tile_pool(name="tpsum", bufs=2, space="PSUM") as tpsum,
                tc.tile_pool(name="gpsum", bufs=2, space="PSUM") as gpsum,
            ):
                for st in range(NS):
                    xs = xstage.tile([P, D], F32)
                    nc.sync.dma_start(out=xs[:], in_=x_d[st * P : (st + 1) * P, :])

                    # transpose x row-tile into bf16 lhsT tiles [d, s]
                    xT = xTp.tile([P, ND, P], BF16)
                    for g in range(NG):
                        tp = tpsum.tile([P, 4 * P], F32)
                        for j in range(4):
                            dt = 4 * g + j
                            nc.tensor.matmul(
                                tp[:, j * P : (j + 1) * P],
                                xs[:, dt * P : (dt + 1) * P],
                                ident[:],
                                is_transpose=True,
                                start=(j == 0),
                                stop=(j == 3),
                            )
                        nc.vector.tensor_copy(xT[:, 4 * g : 4 * (g + 1), :], tp[:])

                    ys = youtp.tile([P, D], F32)
                    for oc in range(NO):
                        gp = gpsum.tile([P, FREE], F32)
                        for dt in range(ND):
                            nc.tensor.matmul(
                                gp[:],
                                xT[:, dt, :],
                                wT[:, dt, oc * FREE : (oc + 1) * FREE],
                                start=(dt == 0),
                                stop=(dt == ND - 1),
                            )
                        nc.vector.tensor_add(
                            ys[:, oc * FREE : (oc + 1) * FREE],
                            gp[:],
                            bb[:, oc * FREE : (oc + 1) * FREE],
                        )
                    nc.sync.dma_start(out=out_d[st * P : (st + 1) * P, :], in_=ys[:])

    nc.compile()
    return nc


_NC_CACHE = None


def _get_nc():
    global _NC_CACHE
    if _NC_CACHE is None:
        _NC_CACHE = build_nc()
    return _NC_CACHE


def make_in_maps(x, W, b, B, A):
    x = np.ascontiguousarray(x, dtype=np.float32)
    W = np.ascontiguousarray(W, dtype=np.float32)
    b = np.ascontiguousarray(b, dtype=np.float32)
    B = np.ascontiguousarray(B, dtype=np.float32)
    A = np.ascontiguousarray(A, dtype=np.float32)
    return [
        {"x": x[i], "W": W, "b": b, "B": B, "A": A} for i in range(N_CORES)
    ]


def run(inputs, **spmd_kwargs):
    """Run the SPMD kernel; returns (output, BassKernelResults)."""
    nc = _get_nc()
    in_maps = make_in_maps(**inputs)
    res = run_bass_kernel_spmd(nc, in_maps, core_ids=list(range(N_CORES)), **spmd_kwargs)
    out = np.stack([res.results[i]["out"] for i in range(N_CORES)]).astype(np.float32)
    return out, res


def kernel(x, W, b, B, A):
    out, _ = run({"x": x, "W": W, "b": b, "B": B, "A": A})
    return out


# revision 4
# speedup vs baseline: 1.1101x; 1.1101x over previous
"""Trainium2 Bass kernel for LoRA linear: y = x @ (W + 2*B@A).T + b.

Full inputs: x (8, 2048, 2048) f32, W (2048, 2048) f32, b (2048,) f32,
B (2048, 16) f32, A (16, 2048) f32.  Output (8, 2048, 2048) f32.

Sharding: data-parallel over the batch dim — core i computes
y[i] = x[i] @ w.T + b with the merged weight w = W + 2*B@A.

Per-core kernel (bf16 TensorEngine compute, f32 accumulate):
  phase 0: cast-DMA A/B to bf16, build 2*B.T via PE transposes,
           broadcast bias, build bf16 identity.
  phase 1: build wT[d, o] = bf16(W.T) + A.T @ (2B).T — bf16 PE transposes
           of cast-DMA'd W tiles (ScalarE evicts PSUM->SBUF), rank-16
           bf16 matmul delta in f32 PSUM added in-place by VectorE.
  phase 2: per 128-row x tile: bf16 PE transposes of the cast-DMA'd
           x tile (ScalarE evicts), then 16x [128,128]x[128,512] bf16
           matmuls per output bank, VectorE adds the bias during
           PSUM->SBUF eviction, DMA out.
"""

import numpy as np

import concourse.bacc as bacc
import concourse.mybir as mybir
import concourse.tile as tile
from concourse import masks
from concourse.bass_utils import run_bass_kernel_spmd

N_CORES = 8
BATCH, S, D = 8, 2048, 2048
RANK = 16
SCALE = 2.0  # alpha / rank = 32 / 16
P = 128  # partitions
FREE = 512  # f32 elems per PSUM bank
ND = D // P  # 16 contraction tiles
NS = S // P  # 16 row tiles per core
NO = D // FREE  # 4 output banks per row tile
NG = ND // 4  # 4 transpose groups (4x 128-col transposes per PSUM bank)

F32 = mybir.dt.float32
BF16 = mybir.dt.bfloat16


def build_nc():
    nc = bacc.Bacc(
        "TRN2", target_bir_lowering=False, debug=False, num_devices=N_CORES
    )
    x_d = nc.dram_tensor("x", [S, D], F32, kind="ExternalInput").ap()
    W_d = nc.dram_tensor("W", [D, D], F32, kind="ExternalInput").ap()
    b_d = nc.dram_tensor("b", [D], F32, kind="ExternalInput").ap()
    B_d = nc.dram_tensor("B", [D, RANK], F32, kind="ExternalInput").ap()
    A_d = nc.dram_tensor("A", [RANK, D], F32, kind="ExternalInput").ap()
    out_d = nc.dram_tensor("out", [S, D], F32, kind="ExternalOutput").ap()

    with tile.TileContext(nc) as tc:
        with (
            tc.tile_pool(name="singles", bufs=1) as singles,
            tc.tile_pool(name="wt", bufs=1) as wtp,
        ):
            ident = singles.tile([P, P], BF16)
            masks.make_identity(nc, ident[:])

            # bias replicated across all 128 partitions
            bb = singles.tile([P, D], F32)
            nc.gpsimd.dma_start(out=bb[:], in_=b_d[None, :].broadcast_to([P, D]))

            A_sb = singles.tile([RANK, D], BF16)
            nc.gpsimd.dma_start(out=A_sb[:], in_=A_d[:])

            # 2 * B.T: cast-load B as [128, (t, r)], PE-transpose, scale
            B2T = singles.tile([RANK, D], BF16)
            Bs = singles.tile([P, ND * RANK], BF16)
            nc.gpsimd.dma_start(
                out=Bs[:], in_=B_d.rearrange("(t p) r -> p t r", p=P)
            )

            # merged transposed weight, bf16: wT[p, dt, o] = w[o, dt*128+p]
            wT = wtp.tile([P, ND, D], BF16)

            # ---- phase 1: build wT = bf16(W.T) + A.T @ (2B).T ----
            with (
                tc.tile_pool(name="wstage", bufs=5) as wstage,
                tc.tile_pool(name="bpsum", bufs=3, space="PSUM") as bpsum,
                tc.tile_pool(name="dpsum", bufs=2, space="PSUM") as dpsum,
            ):
                # 2*B.T from the staged B tiles
                for g in range(NG):
                    bps = bpsum.tile([RANK, 4 * P], BF16, tag="btp")
                    for j in range(4):
                        t = 4 * g + j
                        nc.tensor.matmul(
                            bps[:, j * P : (j + 1) * P],
                            Bs[:, t * RANK : (t + 1) * RANK],
                            ident[:],
                            is_transpose=True,
                            start=(j == 0),
                            stop=(j == 3),
                        )
                    nc.vector.tensor_scalar_mul(
                        B2T[:, g * 4 * P : (g + 1) * 4 * P], bps[:], SCALE
                    )

                for g in range(NG):
                    wst = []
                    for j in range(4):
                        ot = 4 * g + j
                        w_tile = wstage.tile([P, D], BF16, tag="wst")
                        nc.gpsimd.dma_start(
                            out=w_tile[:], in_=W_d[ot * P : (ot + 1) * P, :]
                        )
                        wst.append(w_tile)
                    for dt in range(ND):
                        wslice = wT[:, dt, g * FREE : (g + 1) * FREE]
                        # W.T tiles via bf16 PE transpose, ScalarE evicts
                        ps = bpsum.tile([P, 4 * P], BF16, tag="wtp")
                        for j in range(4):
                            nc.tensor.matmul(
                                ps[:, j * P : (j + 1) * P],
                                wst[j][:, dt * P : (dt + 1) * P],
                                ident[:],
                                is_transpose=True,
                                start=(j == 0),
                                stop=(j == 3),
                            )
                        nc.scalar.copy(wslice, ps[:])
                        # rank-16 LoRA delta, added in place by VectorE
                        dp = dpsum.tile([P, FREE], F32)
                        nc.tensor.matmul(
                            dp[:],
                            A_sb[:, dt * P : (dt + 1) * P],
                            B2T[:, g * FREE : (g + 1) * FREE],
                            start=True,
                            stop=True,
                        )
                        nc.vector.tensor_add(wslice, dp[:], wslice)

            # ---- phase 2: y = x @ wT + b ----
            with (
                tc.tile_pool(name="xstage", bufs=3) as xstage,
                tc.tile_pool(name="xTp", bufs=2) as xTp,
                tc.tile_pool(name="yout", bufs=3) as youtp,
                tc.tile_pool(name="tpsum", bufs=3, space="PSUM") as tpsum,
                tc.tile_pool(name="gpsum", bufs=2, space="PSUM") as gpsum,
            ):
                for st in range(NS):
                    xs = xstage.tile([P, D], BF16)
                    nc.gpsimd.dma_start(
                        out=xs[:], in_=x_d[st * P : (st + 1) * P, :]
                    )

                    # transpose x row-tile into bf16 lhsT tiles [d, s]
                    xT = xTp.tile([P, ND, P], BF16)
                    for g in range(NG):
                        tp = tpsum.tile([P, 4 * P], BF16)
                        for j in range(4):
                            dt = 4 * g + j
                            nc.tensor.matmul(
                                tp[:, j * P : (j + 1) * P],
                                xs[:, dt * P : (dt + 1) * P],
                                ident[:],
                                is_transpose=True,
                                start=(j == 0),
                                stop=(j == 3),
                            )
                        nc.scalar.copy(xT[:, 4 * g : 4 * (g + 1), :], tp[:])

                    ys = youtp.tile([P, D], F32)
                    for oc in range(NO):
                        gp = gpsum.tile([P, FREE], F32)
                        for dt in range(ND):
                            nc.tensor.matmul(
                                gp[:],
                                xT[:, dt, :],
                                wT[:, dt, oc * FREE : (oc + 1) * FREE],
                                start=(dt == 0),
                                stop=(dt == ND - 1),
                            )
                        nc.vector.tensor_add(
                            ys[:, oc * FREE : (oc + 1) * FREE],
                            gp[:],
                            bb[:, oc * FREE : (oc + 1) * FREE],
                        )
                    nc.sync.dma_start(out=out_d[st * P : (st + 1) * P, :], in_=ys[:])

    nc.compile()
    return nc


_NC_CACHE = None


def _get_nc():
    global _NC_CACHE
    if _NC_CACHE is None:
        _NC_CACHE = build_nc()
    return _NC_CACHE


def make_in_maps(x, W, b, B, A):
    x = np.ascontiguousarray(x, dtype=np.float32)
    W = np.ascontiguousarray(W, dtype=np.float32)
    b = np.ascontiguousarray(b, dtype=np.float32)
    B = np.ascontiguousarray(B, dtype=np.float32)
    A = np.ascontiguousarray(A, dtype=np.float32)
    return [
        {"x": x[i], "W": W, "b": b, "B": B, "A": A} for i in range(N_CORES)
    ]


def run(inputs, **spmd_kwargs):
    """Run the SPMD kernel; returns (output, BassKernelResults)."""
    nc = _get_nc()
    in_maps = make_in_maps(**inputs)
    res = run_bass_kernel_spmd(nc, in_maps, core_ids=list(range(N_CORES)), **spmd_kwargs)
    out = np.stack([res.results[i]["out"] for i in range(N_CORES)]).astype(np.float32)
    return out, res


def kernel(x, W, b, B, A):
    out, _ = run({"x": x, "W": W, "b": b, "B": B, "A": A})
    return out


# revision 6
# speedup vs baseline: 1.1891x; 1.0712x over previous
"""Trainium2 Bass kernel for LoRA linear: y = x @ (W + 2*B@A).T + b.

Full inputs: x (8, 2048, 2048) f32, W (2048, 2048) f32, b (2048,) f32,
B (2048, 16) f32, A (16, 2048) f32.  Output (8, 2048, 2048) f32.

Sharding: data-parallel over the batch dim — core i computes
y[i] = x[i] @ w.T + b with the merged weight w = W + 2*B@A.

Per-core kernel (bf16 TensorEngine compute, f32 accumulate):
  phase 0: cast-DMA A/B to bf16, build 2*B.T via PE transposes,
           broadcast bias, build bf16 identity.
  phase 1: build wT[d, o] = bf16(W.T) + A.T @ (2B).T — bf16 PE transposes
           of cast-DMA'd W tiles (ScalarE evicts PSUM->SBUF), rank-16
           bf16 matmul delta in f32 PSUM added in-place by VectorE.
  phase 2: per 128-row x tile: bf16 PE transposes of the cast-DMA'd
           x tile (ScalarE evicts), then 16x [128,128]x[128,512] bf16
           matmuls per output bank, VectorE adds the bias during
           PSUM->SBUF eviction, DMA out.
"""

import numpy as np

import concourse.bacc as bacc
import concourse.mybir as mybir
import concourse.tile as tile
from concourse import masks
from concourse.bass_utils import run_bass_kernel_spmd

N_CORES = 8
BATCH, S, D = 8, 2048, 2048
RANK = 16
SCALE = 2.0  # alpha / rank = 32 / 16
P = 128  # partitions
FREE = 512  # f32 elems per PSUM bank
ND = D // P  # 16 contraction tiles
NS = S // P  # 16 row tiles per core
NO = D // FREE  # 4 output banks per row tile
NG = ND // 4  # 4 transpose groups (4x 128-col transposes per PSUM bank)

F32 = mybir.dt.float32
BF16 = mybir.dt.bfloat16


def build_nc():
    nc = bacc.Bacc(
        "TRN2", target_bir_lowering=False, debug=False, num_devices=N_CORES
    )
    x_d = nc.dram_tensor("x", [S, D], F32, kind="ExternalInput").ap()
    W_d = nc.dram_tensor("W", [D, D], F32, kind="ExternalInput").ap()
    b_d = nc.dram_tensor("b", [D], F32, kind="ExternalInput").ap()
    B_d = nc.dram_tensor("B", [D, RANK], F32, kind="ExternalInput").ap()
    A_d = nc.dram_tensor("A", [RANK, D], F32, kind="ExternalInput").ap()
    out_d = nc.dram_tensor("out", [S, D], F32, kind="ExternalOutput").ap()
    Wb_d = nc.dram_tensor("Wb", [D, D], BF16).ap()  # bf16 W scratch

    with tile.TileContext(nc) as tc:
        with (
            tc.tile_pool(name="singles", bufs=1) as singles,
            tc.tile_pool(name="wt", bufs=1) as wtp,
        ):
            ident = singles.tile([P, P], BF16)
            masks.make_identity(nc, ident[:])

            # bias replicated across all 128 partitions
            bb = singles.tile([P, D], F32)
            nc.gpsimd.dma_start(out=bb[:], in_=b_d[None, :].broadcast_to([P, D]))

            A_sb = singles.tile([RANK, D], BF16)
            nc.gpsimd.dma_start(out=A_sb[:], in_=A_d[:])

            # 2 * B.T: cast-load B as [128, (t, r)], PE-transpose, scale
            B2T = singles.tile([RANK, D], BF16)
            Bs = singles.tile([P, ND * RANK], BF16)
            nc.gpsimd.dma_start(
                out=Bs[:], in_=B_d.rearrange("(t p) r -> p t r", p=P)
            )

            # merged transposed weight, bf16: wT[p, dt, o] = w[o, dt*128+p]
            wT = wtp.tile([P, ND, D], BF16)

            # ---- phase 1: build wT = bf16(W.T) + A.T @ (2B).T ----
            with (
                tc.tile_pool(name="bpsum", bufs=3, space="PSUM") as bpsum,
                tc.tile_pool(name="dpsum", bufs=4, space="PSUM") as dpsum,
            ):
                # 2*B.T from the staged B tiles
                for g in range(NG):
                    bps = bpsum.tile([RANK, 4 * P], BF16, tag="btp")
                    for j in range(4):
                        t = 4 * g + j
                        nc.tensor.matmul(
                            bps[:, j * P : (j + 1) * P],
                            Bs[:, t * RANK : (t + 1) * RANK],
                            ident[:],
                            is_transpose=True,
                            start=(j == 0),
                            stop=(j == 3),
                        )
                    nc.vector.tensor_scalar_mul(
                        B2T[:, g * 4 * P : (g + 1) * 4 * P], bps[:], SCALE
                    )

                # cast W -> bf16 DRAM scratch, column-blocked so the
                # transpose-DMAs can start before the whole cast finishes
                for dt in range(ND):
                    nc.gpsimd.dma_start(
                        out=Wb_d[:, dt * P : (dt + 1) * P],
                        in_=W_d[:, dt * P : (dt + 1) * P],
                    )
                for dt in range(ND):
                    # wT[:, dt, :] = Wb[:, dt*128:(dt+1)*128].T via DMA xbar
                    nc.sync.dma_start_transpose(
                        out=wT[:, dt, :], in_=Wb_d[:, dt * P : (dt + 1) * P]
                    )
                    # rank-16 LoRA delta, added in place by VectorE
                    for g in range(NG):
                        dp = dpsum.tile([P, FREE], F32)
                        nc.tensor.matmul(
                            dp[:],
                            A_sb[:, dt * P : (dt + 1) * P],
                            B2T[:, g * FREE : (g + 1) * FREE],
                            start=True,
                            stop=True,
                        )
                        nc.vector.tensor_add(
                            wT[:, dt, g * FREE : (g + 1) * FREE],
                            dp[:],
                            wT[:, dt, g * FREE : (g + 1) * FREE],
                        )

            # ---- phase 2: y = x @ wT + b ----
            with (
                tc.tile_pool(name="xstage", bufs=3) as xstage,
                tc.tile_pool(name="xTp", bufs=2) as xTp,
                tc.tile_pool(name="yout", bufs=3) as youtp,
                tc.tile_pool(name="tpsum", bufs=3, space="PSUM") as tpsum,
                tc.tile_pool(name="gpsum", bufs=2, space="PSUM") as gpsum,
            ):
                for st in range(NS):
                    xs = xstage.tile([P, D], BF16)
                    nc.gpsimd.dma_start(
                        out=xs[:], in_=x_d[st * P : (st + 1) * P, :]
                    )

                    # transpose x row-tile into bf16 lhsT tiles [d, s]
                    xT = xTp.tile([P, ND, P], BF16)
                    for g in range(NG):
                        tp = tpsum.tile([P, 4 * P], BF16)
                        for j in range(4):
                            dt = 4 * g + j
                            nc.tensor.matmul(
                                tp[:, j * P : (j + 1) * P],
                                xs[:, dt * P : (dt + 1) * P],
                                ident[:],
                                is_transpose=True,
                                start=(j == 0),
                                stop=(j == 3),
                            )
                        nc.scalar.copy(xT[:, 4 * g : 4 * (g + 1), :], tp[:])

                    ys = youtp.tile([P, D], F32)
                    for oc in range(NO):
                        gp = gpsum.tile([P, FREE], F32)
                        for dt in range(ND):
                            nc.tensor.matmul(
                                gp[:],
                                xT[:, dt, :],
                                wT[:, dt, oc * FREE : (oc + 1) * FREE],
                                start=(dt == 0),
                                stop=(dt == ND - 1),
                            )
                        nc.vector.tensor_add(
                            ys[:, oc * FREE : (oc + 1) * FREE],
                            gp[:],
                            bb[:, oc * FREE : (oc + 1) * FREE],
                        )
                    nc.sync.dma_start(out=out_d[st * P : (st + 1) * P, :], in_=ys[:])

    nc.compile()
    return nc


_NC_CACHE = None


def _get_nc():
    global _NC_CACHE
    if _NC_CACHE is None:
        _NC_CACHE = build_nc()
    return _NC_CACHE


def make_in_maps(x, W, b, B, A):
    x = np.ascontiguousarray(x, dtype=np.float32)
    W = np.ascontiguousarray(W, dtype=np.float32)
    b = np.ascontiguousarray(b, dtype=np.float32)
    B = np.ascontiguousarray(B, dtype=np.float32)
    A = np.ascontiguousarray(A, dtype=np.float32)
    return [
        {"x": x[i], "W": W, "b": b, "B": B, "A": A} for i in range(N_CORES)
    ]


def run(inputs, **spmd_kwargs):
    """Run the SPMD kernel; returns (output, BassKernelResults)."""
    nc = _get_nc()
    in_maps = make_in_maps(**inputs)
    res = run_bass_kernel_spmd(nc, in_maps, core_ids=list(range(N_CORES)), **spmd_kwargs)
    out = np.stack([res.results[i]["out"] for i in range(N_CORES)]).astype(np.float32)
    return out, res


def kernel(x, W, b, B, A):
    out, _ = run({"x": x, "W": W, "b": b, "B": B, "A": A})
    return out
